# revision 1
# baseline (speedup 1.0000x reference)
"""Causal self-attention (B=4, T=2048, C=1024, 16 heads) on 8 TRN2 NeuronCores.

Sharding: tensor-parallel over heads. Each core owns 2 heads (128 of the
1024 q/k/v dims): wq/wk/wv are split by rows (output dim), wo by columns.
Each core computes a full [C, B*T] partial of the output projection; the
host sums the 8 partials.

On-core layout is "transposed": activations live as [feature, token] so
every matmul has tokens on the moving free dim (>=256 wide -> float32r
matmuls run at 1 cycle/row). Attention is computed as s^T = K Q^T with
keys on partitions; softmax max-subtraction is skipped (logits are O(10),
exp is safe in fp32) and the denominator comes from a ones-column
appended to V in the P^T @ V matmul. Causal masking replaces masked
probabilities with exp(-10) (the module masks logits with -10, not -inf).
Probabilities and V run in bf16 (denominator and numerator use the same
quantized probs, so the bias largely cancels); projections and scores
stay in f32r.
"""

import os
import sys

import numpy as np

for _p in ("/opt/trn_rl_repo",):
    if _p not in sys.path and os.path.isdir(_p):
        sys.path.insert(0, _p)

_B, _T, _C = 4, 2048, 1024
_NHEAD, _HD = 16, 64
_NC = 8
_LOC = (_NHEAD // _NC) * _HD  # feature dims per core = 128 (2 heads)
_BT = _B * _T                 # 8192 tokens
_TC = 512                     # token chunk (psum bank / moving-operand width)
_NTC = _BT // _TC             # 16 projection chunks
_KC = _C // 128               # 8 contraction chunks over the embedding
_NQC = _T // _TC              # 4 query chunks per batch
_NKB = _T // 128              # 16 key blocks per batch
_EXPM = float(np.exp(-10.0))  # exp of the mask fill value

TRACE = bool(int(os.environ.get("KERNEL_TRACE", "0")))
LAST_EXEC_NS = None
LAST_RESULTS = None

_cache = {}


def _build():
    import concourse.mybir as mybir
    import concourse.tile as tile
    from concourse import bacc

    f32 = mybir.dt.float32
    f32r = mybir.dt.float32r
    bf16 = mybir.dt.bfloat16
    AF = mybir.ActivationFunctionType

    nc = bacc.Bacc("TRN2", target_bir_lowering=False, debug=False)

    xT_d = nc.dram_tensor("xT", [_C, _BT], f32r, kind="ExternalInput").ap()
    wqT_d = nc.dram_tensor("wqT", [_C, _LOC], f32r, kind="ExternalInput").ap()
    wkT_d = nc.dram_tensor("wkT", [_C, _LOC], f32r, kind="ExternalInput").ap()
    wvT_d = nc.dram_tensor("wvT", [_C, _LOC], f32r, kind="ExternalInput").ap()
    woT_d = nc.dram_tensor("woT", [_LOC, _C], f32r, kind="ExternalInput").ap()
    idc_d = nc.dram_tensor("identc", [128, 64], f32r, kind="ExternalInput").ap()
    oneb_d = nc.dram_tensor("onesb", [128, 1], bf16, kind="ExternalInput").ap()
    oner_d = nc.dram_tensor("onesr", [1, 64], f32r, kind="ExternalInput").ap()
    outT_d = nc.dram_tensor("outT", [_C, _BT], f32, kind="ExternalOutput").ap()

    xT_v = xT_d.rearrange("(c p) n -> p c n", p=128)    # [128, 8, 8192]
    wq_v = wqT_d.rearrange("(c p) m -> p c m", p=128)   # [128, 8, 128]
    wk_v = wkT_d.rearrange("(c p) m -> p c m", p=128)
    wv_v = wvT_d.rearrange("(c p) m -> p c m", p=128)
    wo_v = woT_d.rearrange("p (m n) -> p m n", n=128)   # [128, 8, 128]

    with tile.TileContext(nc) as tc:
        with (
            tc.tile_pool(name="consts", bufs=1) as cp,
            tc.tile_pool(name="sb", bufs=2) as sp,
            tc.tile_pool(name="ps", bufs=2, space="PSUM") as pp,
        ):
            # first token chunk of x starts loading before anything else
            xa0 = sp.tile([128, 4, _TC], f32r, tag="xa", bufs=2)
            xb0 = sp.tile([128, 4, _TC], f32r, tag="xb", bufs=2)
            for c4 in range(4):
                nc.sync.dma_start(xa0[:, c4, :], xT_v[:, c4, 0:_TC])
                nc.sync.dma_start(xb0[:, c4, :], xT_v[:, 4 + c4, 0:_TC])
            w_sb = {}
            for nm, v in (("q", wq_v), ("k", wk_v), ("v", wv_v)):
                t = cp.tile([128, _KC, 128], f32r, tag=f"w{nm}")
                nc.sync.dma_start(t[:], v[:])
                w_sb[nm] = t
            wo_sb = cp.tile([128, _KC, 128], f32r, tag="wo")
            nc.sync.dma_start(wo_sb[:], wo_v[:])
            ident = cp.tile([128, 64], f32r, tag="ident")
            nc.sync.dma_start(ident[:], idc_d[:])
            ones1 = cp.tile([65, 64], f32r, tag="ones1")
            nc.sync.dma_start(ones1[64:65, :], oner_d[:])
            ones_b = cp.tile([128, 1], bf16, tag="onesb")
            nc.sync.dma_start(ones_b[:], oneb_d[:])

            qT = cp.tile([128, _BT], f32r, tag="qT")
            kT = cp.tile([128, _BT], f32r, tag="kT")
            # v in [token, dim] layout per 128-token block, per head, with a
            # trailing ones column (row sums -> softmax denominator)
            vaug = cp.tile([128, _BT // 128, 2, 65], bf16, tag="vaug")
            for h in range(2):
                nc.sync.dma_start(
                    vaug[:, :, h, 64:65],
                    oneb_d[:, 0:1].to_broadcast([128, _BT // 128, 1]),
                )

            # ---------------- q/k/v projections ----------------
            for t in range(_NTC):
                tok = slice(t * _TC, (t + 1) * _TC)
                if t == 0:
                    halves = (xa0, xb0)
                else:
                    xa = sp.tile([128, 4, _TC], f32r, tag="xa", bufs=2)
                    xb = sp.tile([128, 4, _TC], f32r, tag="xb", bufs=2)
                    for c4 in range(4):
                        nc.sync.dma_start(xa[:, c4, :], xT_v[:, c4, tok])
                        nc.sync.dma_start(xb[:, c4, :], xT_v[:, 4 + c4, tok])
                    halves = (xa, xb)
                for nm in ("q", "k", "v"):
                    ps = pp.tile([128, _TC], f32, tag="psC", bufs=2)
                    for c in range(_KC):
                        nc.tensor.matmul(
                            ps[:],
                            w_sb[nm][:, c, :],
                            halves[c // 4][:, c % 4, :],
                            start=(c == 0),
                            stop=(c == _KC - 1),
                        )
                    if nm == "q":
                        nc.vector.tensor_copy(qT[:, tok], ps[:])
                    elif nm == "k":
                        nc.vector.tensor_copy(kT[:, tok], ps[:])
                    else:
                        vtc = sp.tile([128, _TC], f32r, tag="vtc", bufs=2)
                        nc.vector.tensor_copy(vtc[:], ps[:])
                        for h in range(2):
                            tp = pp.tile([128, 4, 64], f32, tag="psC",
                                         bufs=2)
                            for s4 in range(4):
                                nc.tensor.transpose(
                                    tp[:, s4, :].bitcast(f32r),
                                    vtc[h * 64:(h + 1) * 64,
                                        s4 * 128:(s4 + 1) * 128],
                                    ident[h * 64:(h + 1) * 64, :],
                                )
                            nc.vector.tensor_copy(
                                vaug[:, t * 4:t * 4 + 4, h, 0:64], tp[:]
                            )

            # ---------------- attention + output projection ----------------
            for b in range(_B):
                ycat = sp.tile([128, _T], f32r, tag="ycat", bufs=2)
                for h in range(2):
                    rows = slice(h * 64, (h + 1) * 64)
                    ytmp = sp.tile([65, _T], f32r, tag="ytmp", bufs=2)
                    # column sums of v over each chunk's fully-masked key
                    # blocks, accumulated in PSUM: suf[:, c] = sum over
                    # kb >= 4c+4 of (v_kb^T @ 1).  Applied (scaled by
                    # exp(-10)) as a bias when copying y out of PSUM.
                    suf_ps = pp.tile([65, _NQC - 1], f32, tag="suf", bufs=1)
                    for c in range(_NQC - 1):
                        for kb in range(4 * c + 4, _NKB):
                            nc.tensor.matmul(
                                suf_ps[:, c:c + 1],
                                vaug[:, b * 16 + kb, h, :],
                                ones_b[:],
                                start=(kb == 4 * c + 4),
                                stop=(kb == _NKB - 1),
                            )
                    suf_sb = sp.tile([65, _NQC - 1], f32, tag="suf", bufs=2)
                    nc.scalar.activation(
                        suf_sb[:], suf_ps[:], AF.Copy, scale=_EXPM
                    )
                    for c in range(_NQC):
                        qc = slice(b * _T + c * _TC, b * _T + (c + 1) * _TC)
                        cc = slice(c * _TC, (c + 1) * _TC)
                        yps = pp.tile([65, _TC], f32, tag="yT", bufs=1)
                        for kb in range(4 * c + 4):
                            j = kb - 4 * c
                            sps = pp.tile([128, _TC], f32, tag="psB",
                                          bufs=4)
                            # band blocks j=1,2: the leading 128j columns
                            # are fully masked, so the score matmul only
                            # needs the tail (tail >= 256 keeps f32r fast)
                            off = 128 * j if j in (1, 2) else 0
                            nc.tensor.matmul(
                                sps[:, off:],
                                kT[rows,
                                   b * _T + kb * 128:
                                   b * _T + (kb + 1) * 128],
                                qT[rows,
                                   b * _T + c * _TC + off:
                                   b * _T + (c + 1) * _TC],
                                start=True, stop=True,
                            )
                            pexp = sp.tile([128, _TC], bf16, tag="pexp",
                                           bufs=16)
                            if j >= 1:
                                # leading 128j columns are fully masked;
                                # the affine_select fills them below
                                nc.scalar.activation(
                                    pexp[:, 128 * j:], sps[:, 128 * j:],
                                    AF.Exp, scale=0.125
                                )
                            else:
                                nc.scalar.activation(
                                    pexp[:], sps[:], AF.Exp, scale=0.125
                                )
                            if j >= 0:
                                # causal: keep where qi - ki - 128j >= 0,
                                # else fill exp(-10); columns right of the
                                # diagonal strip are always valid
                                w = 128 * (j + 1)
                                nc.gpsimd.affine_select(
                                    out=pexp[:, 0:w],
                                    in_=pexp[:, 0:w],
                                    compare_op=mybir.AluOpType.is_ge,
                                    fill=_EXPM,
                                    base=-128 * j,
                                    pattern=[[1, w]],
                                    channel_multiplier=-1,
                                )
                            nc.tensor.matmul(
                                yps[:],
                                vaug[:, b * 16 + kb, h, :],
                                pexp[:],
                                start=(kb == 0),
                                stop=(kb == 4 * c + 3),
                            )
                        if c < _NQC - 1:
                            nc.scalar.activation(
                                ytmp[:, cc], yps[:], AF.Identity,
                                bias=suf_sb[:, c:c + 1],
                            )
                        else:
                            nc.scalar.copy(ytmp[:, cc], yps[:])
                    # normalize: row 64 holds the softmax denominator;
                    # broadcast Z over the 64 dims via a K=1 matmul, then
                    # reciprocal + multiply per chunk
                    for c in range(_NQC):
                        cc = slice(c * _TC, (c + 1) * _TC)
                        zps = pp.tile([64, _TC], f32, tag="psC", bufs=2)
                        nc.tensor.matmul(
                            zps[:],
                            ones1[64:65, :],
                            ytmp[64:65, cc],
                            start=True, stop=True,
                        )
                        zrec = sp.tile([64, _TC], f32, tag="zrec", bufs=2)
                        nc.vector.reciprocal(zrec[:], zps[:])
                        nc.vector.tensor_mul(
                            ycat[rows, cc], ytmp[0:64, cc], zrec[:]
                        )
                # last batch: chunk-outer order starts the output drain
                # as soon as each ycat chunk is normalized
                if b == _B - 1:
                    mc2 = [(m, c2) for c2 in range(_NQC)
                           for m in range(_KC)]
                else:
                    mc2 = [(m, c2) for m in range(_KC)
                           for c2 in range(_NQC)]
                for m, c2 in mc2:
                    ops = pp.tile([128, _TC], f32, tag="psC", bufs=2)
                    nc.tensor.matmul(
                        ops[:],
                        wo_sb[:, m, :],
                        ycat[:, c2 * _TC:(c2 + 1) * _TC],
                        start=True, stop=True,
                    )
                    ostg = sp.tile([128, _TC], f32, tag="ostg", bufs=6)
                    if b == _B - 1 and c2 % 2 == 0:
                        nc.scalar.copy(ostg[:], ops[:])
                    else:
                        nc.vector.tensor_copy(ostg[:], ops[:])
                    nc.sync.dma_start(
                        outT_d[m * 128:(m + 1) * 128,
                               b * _T + c2 * _TC:b * _T + (c2 + 1) * _TC],
                        ostg[:],
                    )

    nc.compile()
    return nc, outT_d.name


def _get_nc():
    if "nc" not in _cache:
        _cache["nc"] = _build()
    return _cache["nc"]


def kernel(**inputs):
    import ml_dtypes

    from concourse.bass_utils import run_bass_kernel_spmd

    x = np.ascontiguousarray(np.asarray(inputs["x"]), dtype=np.float32)
    wq = np.ascontiguousarray(np.asarray(inputs["wq"]), dtype=np.float32)
    wk = np.ascontiguousarray(np.asarray(inputs["wk"]), dtype=np.float32)
    wv = np.ascontiguousarray(np.asarray(inputs["wv"]), dtype=np.float32)
    wo = np.ascontiguousarray(np.asarray(inputs["wo"]), dtype=np.float32)

    xT = np.ascontiguousarray(x.reshape(_BT, _C).T)
    identc = np.zeros((128, 64), dtype=np.float32)
    identc[np.arange(128), np.arange(128) % 64] = 1.0
    onesb = np.ones((128, 1), dtype=ml_dtypes.bfloat16)
    onesr = np.ones((1, 64), dtype=np.float32)

    in_maps = []
    for i in range(_NC):
        r = slice(_LOC * i, _LOC * (i + 1))
        in_maps.append({
            "xT": xT,
            "wqT": np.ascontiguousarray(wq[r].T),
            "wkT": np.ascontiguousarray(wk[r].T),
            "wvT": np.ascontiguousarray(wv[r].T),
            "woT": np.ascontiguousarray(wo[:, r].T),
            "identc": identc,
            "onesb": onesb,
            "onesr": onesr,
        })

    nc, outname = _get_nc()
    try:
        res = run_bass_kernel_spmd(nc, in_maps, list(range(_NC)), trace=TRACE)
    except ModuleNotFoundError:
        # NTFF profiling hook unavailable in this container
        res = run_bass_kernel_spmd(nc, in_maps, list(range(_NC)), trace=False)

    global LAST_EXEC_NS, LAST_RESULTS
    LAST_EXEC_NS = res.exec_time_ns
    LAST_RESULTS = res

    acc = np.zeros((_C, _BT), dtype=np.float64)
    for i in range(_NC):
        acc += res.results[i][outname]
    return np.ascontiguousarray(acc.T).reshape(_B, _T, _C).astype(np.float32)



# revision 42
# speedup vs baseline: 1.1688x; 1.1688x over previous
"""Causal self-attention (B=4, T=2048, C=1024, 16 heads) on 8 TRN2 NeuronCores.

Sharding: tensor-parallel over heads. Each core owns 2 heads (128 of the
1024 q/k/v dims): wq/wk/wv are split by rows (output dim), wo by columns.
Each core computes a full [C, B*T] partial of the output projection; the
host sums the 8 partials.

All-bf16 dataflow: x, weights, q/k/v and probabilities are bf16 (the
moving operand's dtype sets matmul speed: bf16 = 1 cycle/row at any
width). Scores s^T = K Q^T land in PSUM f32; exp (scale 1/8) runs on the
scalar engine; causal masking on the diagonal 128-blocks uses gpsimd
affine_select with fill exp(-10) (the module masks logits with -10, not
-inf). The P@V matmul is *flipped*: stationary = pexp [128k x 128q]
slice, moving = v-block [128k x 65] (64 dims + a ones column yielding the
softmax denominator) -> 65-cycle matmuls accumulating y [128q x 65] per
128-query tile. Fully-masked key blocks enter via a PSUM-init matmul:
y starts from exp(-10) * (suffix sums of per-block V column sums),
computed with a one-hot stationary trick plus a constant suffix-mask
matrix. V is projected directly into [token, dim] layout (stationary =
x chunk, moving = wv) so no separate transpose pass is needed. Each y
tile is normalized per-partition (reciprocal of its ones column),
transposed back to [dim, token] with one 128x128 PE transpose, and
projected through wo. Projection and output-projection matmuls are
chopped into ~0.4us quanta and pumped between attention blocks so the
tensor engine stays busy while the scalar engine works through the exp
chain; partial outputs stream out in bf16.
"""

import os
import sys
from collections import deque

import numpy as np

for _p in ("/opt/trn_rl_repo",):
    if _p not in sys.path and os.path.isdir(_p):
        sys.path.insert(0, _p)

_B, _T, _C = 4, 2048, 1024
_NHEAD, _HD = 16, 64
_NC = 8
_LOC = (_NHEAD // _NC) * _HD  # feature dims per core = 128 (2 heads)
_BT = _B * _T                 # 8192 tokens
_TC = 512                     # token chunk (psum bank / moving width)
_KC = _C // 128               # 8 contraction chunks over the embedding
_NQC = _T // _TC              # 4 query chunks per batch
_NKB = _T // 128              # 16 key blocks per batch
_EXPM = float(np.exp(-10.0))  # exp of the mask fill value

TRACE = bool(int(os.environ.get("KERNEL_TRACE", "0")))
LAST_EXEC_NS = None
LAST_RESULTS = None

_cache = {}


def _build():
    import concourse.mybir as mybir
    import concourse.tile as tile
    from concourse import bacc

    f32 = mybir.dt.float32
    f32r = mybir.dt.float32r
    bf16 = mybir.dt.bfloat16
    AF = mybir.ActivationFunctionType

    nc = bacc.Bacc("TRN2", target_bir_lowering=False, debug=False)

    xT_d = nc.dram_tensor("xT", [_C, _BT], bf16, kind="ExternalInput").ap()
    wqT_d = nc.dram_tensor("wqT", [_C, _LOC], bf16, kind="ExternalInput").ap()
    wkT_d = nc.dram_tensor("wkT", [_C, _LOC], bf16, kind="ExternalInput").ap()
    wvT_d = nc.dram_tensor("wvT", [_C, _LOC], bf16, kind="ExternalInput").ap()
    woT_d = nc.dram_tensor("woT", [_LOC, _C], bf16, kind="ExternalInput").ap()
    mbig_d = nc.dram_tensor("mbig", [128, 16 * 128], bf16,
                            kind="ExternalInput").ap()
    oneh_d = nc.dram_tensor("onehot", [128, 16 * 128], bf16,
                            kind="ExternalInput").ap()
    idr_d = nc.dram_tensor("identr", [128, 128], f32r,
                           kind="ExternalInput").ap()
    outT_d = nc.dram_tensor("outT", [_C, _BT], bf16,
                            kind="ExternalOutput").ap()

    xT_v = xT_d.rearrange("(c p) n -> p c n", p=128)    # [128, 8, 8192]
    wq_v = wqT_d.rearrange("(c p) m -> p c m", p=128)   # [128, 8, 128]
    wk_v = wkT_d.rearrange("(c p) m -> p c m", p=128)
    wv_v = wvT_d.rearrange("(c p) m -> p c m", p=128)
    wo_v = woT_d.rearrange("p (m n) -> p m n", n=128)   # [128, 8, 128]

    with tile.TileContext(nc) as tc:
        with (
            tc.tile_pool(name="consts", bufs=1) as cp,
            tc.tile_pool(name="sb", bufs=2) as sp,
            tc.tile_pool(name="ps", bufs=2, space="PSUM") as pp,
        ):
            # ---------------- constants ----------------
            # (x chunk 0 is DMA'd before the weights -- see main flow --
            # so the first projection matmul can start ~6us earlier)
            w_sb = {}
            for nm in ("q", "k", "v"):
                wt = cp.tile([128, _KC, 128], bf16, tag=f"w{nm}",
                             name=f"w{nm}")
                w_sb[nm] = wt
            wo_sb = cp.tile([128, _KC, 128], bf16, tag="wo")
            mbig = cp.tile([128, 16 * 128], bf16, tag="mbig")
            oneh = cp.tile([128, 16 * 128], bf16, tag="oneh")
            identr = cp.tile([128, 128], f32r, tag="identr")

            def load_consts():
                for nm, v in (("q", wq_v), ("k", wk_v), ("v", wv_v)):
                    nc.sync.dma_start(w_sb[nm][:], v[:])
                nc.sync.dma_start(wo_sb[:], wo_v[:])
                nc.sync.dma_start(mbig[:], mbig_d[:])
                nc.sync.dma_start(oneh[:], oneh_d[:])
                nc.sync.dma_start(identr[:], idr_d[:])

            # per-batch activation tiles (written during prev batch's attn)
            def batch_tiles():
                qTb = sp.tile([128, _T], bf16, tag="qTb", bufs=2)
                kTb = sp.tile([128, _T], bf16, tag="kTb", bufs=2)
                # v in [token, dim] layout per 128-token key block:
                # [.., kb, h, 0:64] = v dims, [.., kb, h, 64] = ones
                vab = sp.tile([128, _NKB, 2, 65], bf16, tag="vab", bufs=2)
                nc.vector.memset(vab[:, :, :, 64:65], 1.0)
                return {"q": qTb, "k": kTb, "v": vab}

            # ---------------- interleaved work quanta ----------------
            # Projection chunks (512 tokens) and output-projection drains
            # are emitted as generator steps (~0.4us of PE work each) and
            # pumped from inside the attention loops, keeping the tensor
            # engine busy while the scalar engine runs the exp chain.
            pending = deque()
            sched = {"q": 0, "w": 0}  # pending quanta / remaining weight

            def push_gen(g, n):
                pending.append(g)
                sched["q"] += n

            def pump(n=1):
                for _ in range(n):
                    while pending:
                        try:
                            next(pending[0])
                        except StopIteration:
                            pending.popleft()
                            continue
                        sched["q"] -= 1
                        break
                    else:
                        return

            def pump_w(w):
                # proportional pumping: spread the pending quanta evenly
                # over the remaining weighted slots of this batch
                W = max(sched["w"], 1)
                n = min(sched["q"], -(-sched["q"] * w // W))
                sched["w"] = max(sched["w"] - w, 0)
                pump(n)

            def drain():
                while pending:
                    pump(1)

            def proj_gen(t, bt, xall):
                lt = t % 4
                tok = slice(lt * _TC, (lt + 1) * _TC)
                for nm in ("q", "k"):
                    ps = pp.tile([128, _TC], f32, tag="pbig", bufs=2)
                    for c in range(_KC):
                        nc.tensor.matmul(
                            ps[:],
                            w_sb[nm][:, c, :],
                            xall[:, c, :],
                            start=(c == 0),
                            stop=(c == _KC - 1),
                        )
                        if c % 2 == 1:
                            yield
                    nc.vector.tensor_copy(bt[nm][:, tok], ps[:])
                    yield
                # direct v^T: stationary = x slice, moving = wv chunk ->
                # psum [128 tok, 2, 64] per 128-token block
                ps = pp.tile([128, 4, 2, 64], f32, tag="pbig", bufs=2)
                for tb in range(4):
                    for c in range(_KC):
                        nc.tensor.matmul(
                            ps[:, tb, :, :],
                            xall[:, c, tb * 128:(tb + 1) * 128],
                            w_sb["v"][:, c, :],
                            start=(c == 0),
                            stop=(c == _KC - 1),
                        )
                    yield
                kb0 = lt * 4
                nc.vector.tensor_copy(
                    bt["v"][:, kb0:kb0 + 4, :, 0:64], ps[:])
                yield

            def push_chunk(t, bt):
                lt = t % 4
                lo = (t // 4) * _T + lt * _TC
                xall = sp.tile([128, _KC, _TC], bf16, tag="xall", bufs=2)
                nc.sync.dma_start(xall[:], xT_v[:, :, lo:lo + _TC])
                push_gen(proj_gen(t, bt, xall), 15)

            def outproj_gen(b, ycat, ms, c2s, tail):
                for m in ms:
                    ostg = sp.tile([128, len(c2s) * _TC], bf16, tag="ostg",
                                   bufs=3)
                    for i, c2 in enumerate(c2s):
                        ops = pp.tile([128, _TC], f32, tag="pbig", bufs=2)
                        nc.tensor.matmul(
                            ops[:],
                            wo_sb[:, m, :],
                            ycat[:, c2 * _TC:(c2 + 1) * _TC],
                            start=True, stop=True,
                        )
                        # rotate the psum->sbuf drain across engines
                        # (gpsimd can't read PSUM; keep Act mostly free
                        # for the exp chain)
                        dst = ostg[:, i * _TC:(i + 1) * _TC]
                        if tail and m % 2 == 0:
                            nc.scalar.copy(dst, ops[:])
                        else:
                            nc.vector.tensor_copy(dst, ops[:])
                        yield
                    lo = b * _T + c2s[0] * _TC
                    nc.sync.dma_start(
                        outT_d[m * 128:(m + 1) * 128,
                               lo:lo + len(c2s) * _TC],
                        ostg[:],
                    )

            # ---------------- attention ----------------
            def attn_batch(b, bt, bt_next):
                qTb, kTb, vab = bt["q"], bt["k"], bt["v"]
                sched["w"] = _NQC * (8 * 2 + 4)
                # per-block V column sums (both heads, incl. ones cols)
                # via one-hot stationaries stacked into psum rows 0..15
                # full-128-partition one-hot stationaries: narrower
                # matmuls silently produce zeros on this toolchain
                cps = pp.tile([128, _TC], f32, tag="sps", bufs=4)
                for kb in range(_NKB):
                    nc.tensor.matmul(
                        cps[:, 0:130],
                        oneh[:, kb * 128:(kb + 1) * 128],
                        vab[:, kb, :, :],
                        start=(kb == 0),
                        stop=(kb == _NKB - 1),
                    )
                csum = sp.tile([128, 130], bf16, tag="csum", bufs=2)
                nc.vector.tensor_copy(csum[:], cps[:, 0:130])

                def emit_score(cc, kb):
                    # score block + exp (+ causal mask on diagonal
                    # blocks); returns the pexp tiles per head
                    j = kb - 4 * cc
                    off = 128 * j if j > 0 else 0
                    out = {}
                    for h in range(2):
                        rows = slice(h * 64, (h + 1) * 64)
                        sps = pp.tile([128, _TC], f32, tag="sps", bufs=4)
                        nc.tensor.matmul(
                            sps[:, off:],
                            kTb[rows, kb * 128:(kb + 1) * 128],
                            qTb[rows, cc * _TC + off:(cc + 1) * _TC],
                            start=True, stop=True,
                        )
                        pexp = sp.tile([128, _TC], bf16, tag="pexp",
                                       bufs=30)
                        nc.scalar.activation(
                            pexp[:, off:], sps[:, off:], AF.Exp,
                            scale=0.125,
                        )
                        if j >= 0:
                            # diagonal 128-block: keep where q >= k,
                            # else fill exp(-10)
                            nc.gpsimd.affine_select(
                                out=pexp[:, 128 * j:128 * (j + 1)],
                                in_=pexp[:, 128 * j:128 * (j + 1)],
                                compare_op=mybir.AluOpType.is_ge,
                                fill=_EXPM,
                                base=0,
                                pattern=[[1, 128]],
                                channel_multiplier=-1,
                            )
                        out[h] = pexp
                    return out

                ycat = sp.tile([128, _T], bf16, tag="ycat", bufs=2)
                # chunk-pipelined: scores/exp for chunk c+1 are emitted
                # while chunk c's PV groups accumulate, so the exp chain
                # is a full chunk ahead of its consumers.  PSUM start
                # flags zero the whole 2KB bank, so each (h, u) group
                # must run contiguously: init + PVs + stop, one group
                # live per bank at a time.
                pex = {kb: emit_score(0, kb) for kb in range(4)}
                for c in range(_NQC):
                    nkb = 4 * c + 4
                    nxt = list(range(nkb + 4)) if c < 3 else []
                    pex_next = {}
                    emitted = 0
                    y2 = []
                    for h in range(2):
                        t = pp.tile([128, 4, 128], f32, tag="y2", bufs=2,
                                    name=f"y2_{c}_{h}")
                        y2.append(t)
                    for u in range(4):
                        qt = 4 * c + u
                        for h in range(2):
                            nc.tensor.matmul(
                                y2[h][:, u, 0:65],
                                mbig[:, qt * 128:(qt + 1) * 128],
                                csum[:, h * 65:h * 65 + 65],
                                start=True, stop=False,
                            )
                            for kb in range(4 * c + u + 1):
                                nc.tensor.matmul(
                                    y2[h][:, u, 0:65],
                                    pex[kb][h][:, u * 128:(u + 1) * 128],
                                    vab[:, kb, h, :],
                                    start=False,
                                    stop=(kb == 4 * c + u),
                                )
                            share = (len(nxt) * (2 * u + h + 1)) // 8
                            while emitted < share:
                                pex_next[nxt[emitted]] = emit_score(
                                    c + 1, nxt[emitted])
                                emitted += 1
                            pump_w(2)
                        # query tile u complete: normalize by its Z
                        # column, transpose to [dim, tok]
                        ynorm = sp.tile([128, 128], f32r, tag="yn",
                                        bufs=4)
                        for h2 in range(2):
                            zr = sp.tile([128, 1], f32, tag="zr", bufs=4)
                            nc.vector.reciprocal(
                                zr[:], y2[h2][:, u, 64:65])
                            nc.vector.tensor_scalar_mul(
                                ynorm[:, h2 * 64:(h2 + 1) * 64],
                                y2[h2][:, u, 0:64],
                                zr[:],
                            )
                        yt = pp.tile([128, _TC], f32, tag="sps", bufs=4)
                        nc.tensor.transpose(
                            yt[:, 0:128].bitcast(f32r),
                            ynorm[:],
                            identr[:],
                        )
                        nc.vector.tensor_copy(
                            ycat[:, c * _TC + u * 128:
                                 c * _TC + (u + 1) * 128],
                            yt[:, 0:128],
                        )
                        pump_w(1)
                    pex = pex_next
                    if b == _B - 1:
                        # drain this chunk through the output projection;
                        # all but the last chunk spread into the next
                        # chunk's pump slots
                        g = outproj_gen(b, ycat, range(_KC), [c], True)
                        if c == _NQC - 1:
                            for _ in g:
                                pass
                        else:
                            push_gen(g, _KC)
                    if bt_next is not None and c < 3:
                        push_chunk(4 * (b + 1) + c + 1, bt_next)
                return ycat

            # ---------------- main flow ----------------
            bt = batch_tiles()
            push_chunk(0, bt)
            load_consts()
            for t in range(1, 4):
                push_chunk(t, bt)
            drain()
            for b in range(_B):
                bt_next = batch_tiles() if b < _B - 1 else None
                if bt_next is not None:
                    push_chunk(4 * (b + 1), bt_next)
                ycat = attn_batch(b, bt, bt_next)
                # everything feeding attn(b+1) must be emitted before
                # attn(b+1) starts (in-order engines); normally a no-op
                drain()
                if b < _B - 1:
                    push_gen(
                        outproj_gen(b, ycat, range(_KC), [0, 1, 2, 3],
                                    False),
                        _KC * _NQC)
                bt = bt_next
            drain()

    nc.compile()
    return nc, outT_d.name


def _get_nc():
    if "nc" not in _cache:
        _cache["nc"] = _build()
    return _cache["nc"]


def _consts():
    import ml_dtypes

    mbig = np.zeros((128, 16 * 128), dtype=np.float32)
    for kb in range(16):
        for qt in range(16):
            if kb > qt:
                mbig[kb, qt * 128:(qt + 1) * 128] = _EXPM
    onehot = np.zeros((128, 16 * 128), dtype=np.float32)
    for kb in range(16):
        onehot[:, kb * 128 + kb] = 1.0
    identr = np.eye(128, dtype=np.float32)
    return (
        mbig.astype(ml_dtypes.bfloat16),
        onehot.astype(ml_dtypes.bfloat16),
        identr,
    )


def kernel(**inputs):
    import ml_dtypes

    from concourse.bass_utils import run_bass_kernel_spmd

    x = np.ascontiguousarray(np.asarray(inputs["x"]), dtype=np.float32)
    wq = np.ascontiguousarray(np.asarray(inputs["wq"]), dtype=np.float32)
    wk = np.ascontiguousarray(np.asarray(inputs["wk"]), dtype=np.float32)
    wv = np.ascontiguousarray(np.asarray(inputs["wv"]), dtype=np.float32)
    wo = np.ascontiguousarray(np.asarray(inputs["wo"]), dtype=np.float32)

    bf16 = ml_dtypes.bfloat16
    xT = np.ascontiguousarray(x.reshape(_BT, _C).T).astype(bf16)
    mbig, onehot, identr = _consts()

    in_maps = []
    for i in range(_NC):
        r = slice(_LOC * i, _LOC * (i + 1))
        in_maps.append({
            "xT": xT,
            "wqT": np.ascontiguousarray(wq[r].T).astype(bf16),
            "wkT": np.ascontiguousarray(wk[r].T).astype(bf16),
            "wvT": np.ascontiguousarray(wv[r].T).astype(bf16),
            "woT": np.ascontiguousarray(wo[:, r].T).astype(bf16),
            "mbig": mbig,
            "onehot": onehot,
            "identr": identr,
        })

    nc, outname = _get_nc()
    try:
        res = run_bass_kernel_spmd(nc, in_maps, list(range(_NC)), trace=TRACE)
    except ModuleNotFoundError:
        # NTFF profiling hook unavailable in this container
        res = run_bass_kernel_spmd(nc, in_maps, list(range(_NC)), trace=False)

    global LAST_EXEC_NS, LAST_RESULTS
    LAST_EXEC_NS = res.exec_time_ns
    LAST_RESULTS = res

    acc = np.zeros((_C, _BT), dtype=np.float64)
    for i in range(_NC):
        acc += np.asarray(res.results[i][outname], dtype=np.float64)
    return np.ascontiguousarray(acc.T).reshape(_B, _T, _C).astype(np.float32)


# revision 50
# speedup vs baseline: 1.1808x; 1.0102x over previous
"""Causal self-attention (B=4, T=2048, C=1024, 16 heads) on 8 TRN2 NeuronCores.

Sharding: tensor-parallel over heads. Each core owns 2 heads (128 of the
1024 q/k/v dims): wq/wk/wv are split by rows (output dim), wo by columns.
Each core computes a full [C, B*T] partial of the output projection; the
host sums the 8 partials.

All-bf16 dataflow: x, weights, q/k/v and probabilities are bf16 (the
moving operand's dtype sets matmul speed: bf16 = 1 cycle/row at any
width). Scores s^T = K Q^T land in PSUM f32; exp (scale 1/8) runs on the
scalar engine; causal masking on the diagonal 128-blocks uses gpsimd
affine_select with fill exp(-10) (the module masks logits with -10, not
-inf). The P@V matmul is *flipped*: stationary = pexp [128k x 128q]
slice, moving = v-block [128k x 65] (64 dims + a ones column yielding the
softmax denominator) -> 65-cycle matmuls accumulating y [128q x 65] per
128-query tile. Fully-masked key blocks enter via a PSUM-init matmul:
y starts from exp(-10) * (suffix sums of per-block V column sums),
computed with a one-hot stationary trick plus a constant suffix-mask
matrix. V is projected directly into [token, dim] layout (stationary =
x chunk, moving = wv) so no separate transpose pass is needed. Each y
tile is normalized per-partition (reciprocal of its ones column),
transposed back to [dim, token] with one 128x128 PE transpose, and
projected through wo. Projection and output-projection matmuls are
chopped into ~0.4us quanta and pumped between attention blocks so the
tensor engine stays busy while the scalar engine works through the exp
chain; partial outputs stream out in bf16.
"""

import os
import sys
from collections import deque

import numpy as np

for _p in ("/opt/trn_rl_repo",):
    if _p not in sys.path and os.path.isdir(_p):
        sys.path.insert(0, _p)

_B, _T, _C = 4, 2048, 1024
_NHEAD, _HD = 16, 64
_NC = 8
_LOC = (_NHEAD // _NC) * _HD  # feature dims per core = 128 (2 heads)
_BT = _B * _T                 # 8192 tokens
_TC = 512                     # token chunk (psum bank / moving width)
_KC = _C // 128               # 8 contraction chunks over the embedding
_NQC = _T // _TC              # 4 query chunks per batch
_NKB = _T // 128              # 16 key blocks per batch
_EXPM = float(np.exp(-10.0))  # exp of the mask fill value

TRACE = bool(int(os.environ.get("KERNEL_TRACE", "0")))
LAST_EXEC_NS = None
LAST_RESULTS = None

_cache = {}


def _build():
    import concourse.mybir as mybir
    import concourse.tile as tile
    from concourse import bacc

    f32 = mybir.dt.float32
    f32r = mybir.dt.float32r
    bf16 = mybir.dt.bfloat16
    AF = mybir.ActivationFunctionType

    nc = bacc.Bacc("TRN2", target_bir_lowering=False, debug=False)

    xT_d = nc.dram_tensor("xT", [_C, _BT], bf16, kind="ExternalInput").ap()
    wqT_d = nc.dram_tensor("wqT", [_C, _LOC], bf16, kind="ExternalInput").ap()
    wkT_d = nc.dram_tensor("wkT", [_C, _LOC], bf16, kind="ExternalInput").ap()
    wvT_d = nc.dram_tensor("wvT", [_C, _LOC], bf16, kind="ExternalInput").ap()
    woT_d = nc.dram_tensor("woT", [_LOC, _C], bf16, kind="ExternalInput").ap()
    mbig_d = nc.dram_tensor("mbig", [128, 16 * 128], bf16,
                            kind="ExternalInput").ap()
    oneh_d = nc.dram_tensor("onehot", [128, 16 * 128], bf16,
                            kind="ExternalInput").ap()
    idr_d = nc.dram_tensor("identr", [128, 128], f32r,
                           kind="ExternalInput").ap()
    outT_d = nc.dram_tensor("outT", [_BT, _C], bf16,
                            kind="ExternalOutput").ap()

    xT_v = xT_d.rearrange("(c p) n -> p c n", p=128)    # [128, 8, 8192]
    wq_v = wqT_d.rearrange("(c p) m -> p c m", p=128)   # [128, 8, 128]
    wk_v = wkT_d.rearrange("(c p) m -> p c m", p=128)
    wv_v = wvT_d.rearrange("(c p) m -> p c m", p=128)
    wo_v = woT_d.rearrange("p (m n) -> p m n", n=128)   # [128, 8, 128]

    with tile.TileContext(nc) as tc:
        with (
            tc.tile_pool(name="consts", bufs=1) as cp,
            tc.tile_pool(name="sb", bufs=2) as sp,
            tc.tile_pool(name="ps", bufs=2, space="PSUM") as pp,
        ):
            # ---------------- constants ----------------
            # (x chunk 0 is DMA'd before the weights -- see main flow --
            # so the first projection matmul can start ~6us earlier)
            w_sb = {}
            for nm in ("q", "k", "v"):
                wt = cp.tile([128, _KC, 128], bf16, tag=f"w{nm}",
                             name=f"w{nm}")
                w_sb[nm] = wt
            wo_sb = cp.tile([128, _KC, 128], bf16, tag="wo")
            mbig = cp.tile([128, 16 * 128], bf16, tag="mbig")
            oneh = cp.tile([128, 16 * 128], bf16, tag="oneh")
            identr = cp.tile([128, 128], f32r, tag="identr")

            def load_consts():
                for nm, v in (("k", wk_v), ("v", wv_v)):
                    nc.sync.dma_start(w_sb[nm][:], v[:])
                nc.sync.dma_start(wo_sb[:], wo_v[:])
                nc.sync.dma_start(mbig[:], mbig_d[:])
                nc.sync.dma_start(oneh[:], oneh_d[:])
                nc.sync.dma_start(identr[:], idr_d[:])

            # per-batch activation tiles (written during prev batch's attn)
            def batch_tiles():
                qTb = sp.tile([128, _T], bf16, tag="qTb", bufs=2)
                kTb = sp.tile([128, _T], bf16, tag="kTb", bufs=2)
                # v in [token, dim] layout per 128-token key block:
                # [.., kb, h, 0:64] = v dims, [.., kb, h, 64] = ones
                vab = sp.tile([128, _NKB, 2, 65], bf16, tag="vab", bufs=2)
                nc.vector.memset(vab[:, :, :, 64:65], 1.0)
                return {"q": qTb, "k": kTb, "v": vab}

            # ---------------- interleaved work quanta ----------------
            # Projection chunks (512 tokens) and output-projection drains
            # are emitted as generator steps (~0.4us of PE work each) and
            # pumped from inside the attention loops, keeping the tensor
            # engine busy while the scalar engine runs the exp chain.
            pending = deque()
            sched = {"q": 0, "w": 0}  # pending quanta / remaining weight

            def push_gen(g, n):
                pending.append(g)
                sched["q"] += n

            def pump(n=1):
                for _ in range(n):
                    while pending:
                        try:
                            next(pending[0])
                        except StopIteration:
                            pending.popleft()
                            continue
                        sched["q"] -= 1
                        break
                    else:
                        return

            def pump_w(w):
                # proportional pumping with error diffusion: spread the
                # pending quanta evenly over the remaining weighted slots
                # of this batch (ceil would drain the queue early and
                # leave the last chunk dry)
                W = max(sched["w"], 1)
                sched["c"] = sched.get("c", 0.0) + sched["q"] * w / W
                n = min(sched["q"], int(sched["c"]))
                sched["c"] -= n
                sched["w"] = max(sched["w"] - w, 0)
                pump(n)

            def drain():
                while pending:
                    pump(1)

            def proj_gen(t, bt, xall):
                lt = t % 4
                tok = slice(lt * _TC, (lt + 1) * _TC)
                for nm in ("q", "k"):
                    ps = pp.tile([128, _TC], f32, tag="pbig", bufs=2)
                    for c in range(_KC):
                        nc.tensor.matmul(
                            ps[:],
                            w_sb[nm][:, c, :],
                            xall[:, c, :],
                            start=(c == 0),
                            stop=(c == _KC - 1),
                        )
                        if c % 2 == 1:
                            yield
                    nc.vector.tensor_copy(bt[nm][:, tok], ps[:])
                    yield
                # direct v^T: stationary = x slice, moving = wv chunk ->
                # psum [128 tok, 2, 64] per 128-token block
                ps = pp.tile([128, 4, 2, 64], f32, tag="pbig", bufs=2)
                for tb in range(4):
                    for c in range(_KC):
                        nc.tensor.matmul(
                            ps[:, tb, :, :],
                            xall[:, c, tb * 128:(tb + 1) * 128],
                            w_sb["v"][:, c, :],
                            start=(c == 0),
                            stop=(c == _KC - 1),
                        )
                    yield
                kb0 = lt * 4
                nc.vector.tensor_copy(
                    bt["v"][:, kb0:kb0 + 4, :, 0:64], ps[:])
                yield

            def push_chunk(t, bt):
                lt = t % 4
                lo = (t // 4) * _T + lt * _TC
                xall = sp.tile([128, _KC, _TC], bf16, tag="xall", bufs=2)
                nc.sync.dma_start(xall[:, 0:4, :],
                                  xT_v[:, 0:4, lo:lo + _TC])
                nc.sync.dma_start(xall[:, 4:8, :],
                                  xT_v[:, 4:8, lo:lo + _TC])
                push_gen(proj_gen(t, bt, xall), 15)

            def make_post(b, c, u, y2, last):
                # deferred normalize -> transpose -> token-major output
                # projection for one 128-query tile (run one tile late so
                # the cross-engine latency hides under the next group)
                def run():
                    ynorm = sp.tile([128, 128], f32r, tag="yn", bufs=4)
                    for h2 in range(2):
                        zr = sp.tile([128, 1], f32, tag="zr", bufs=4)
                        nc.vector.reciprocal(zr[:], y2[h2][:, u, 64:65])
                        nc.vector.tensor_scalar_mul(
                            ynorm[:, h2 * 64:(h2 + 1) * 64],
                            y2[h2][:, u, 0:64],
                            zr[:],
                        )
                    yt = pp.tile([128, _TC], f32, tag="sps", bufs=4)
                    nc.tensor.transpose(
                        yt[:, 0:128].bitcast(f32r), ynorm[:], identr[:])
                    yts = sp.tile([128, 128], bf16, tag="yts", bufs=4)
                    nc.vector.tensor_copy(yts[:], yt[:, 0:128])
                    og = sp.tile([128, 2, _TC], bf16, tag="og", bufs=4)
                    for half in range(2):
                        ops = pp.tile([128, _TC], f32, tag="pbig", bufs=2)
                        nc.tensor.matmul(
                            ops[:],
                            yts[:],
                            wo_sb[:, 4 * half:4 * half + 4, :],
                            start=True, stop=True,
                        )
                        if last and half == 0:
                            nc.scalar.copy(og[:, half, :], ops[:])
                        else:
                            nc.vector.tensor_copy(og[:, half, :], ops[:])
                    tokr = b * _T + c * _TC + u * 128
                    nc.sync.dma_start(outT_d[tokr:tokr + 128, :], og[:])
                return run

            # ---------------- attention ----------------
            def attn_batch(b, bt, bt_next):
                qTb, kTb, vab = bt["q"], bt["k"], bt["v"]
                sched["w"] = _NQC * (8 * 2 + 4)
                # per-block V column sums (both heads, incl. ones cols)
                # via one-hot stationaries stacked into psum rows 0..15
                # full-128-partition one-hot stationaries: narrower
                # matmuls silently produce zeros on this toolchain
                cps = pp.tile([128, _TC], f32, tag="sps", bufs=4)
                for kb in range(_NKB):
                    nc.tensor.matmul(
                        cps[:, 0:130],
                        oneh[:, kb * 128:(kb + 1) * 128],
                        vab[:, kb, :, :],
                        start=(kb == 0),
                        stop=(kb == _NKB - 1),
                    )
                csum = sp.tile([128, 130], bf16, tag="csum", bufs=2)
                nc.vector.tensor_copy(csum[:], cps[:, 0:130])

                def emit_score(cc, kb):
                    # score block + exp (+ causal mask on diagonal
                    # blocks); returns the pexp tiles per head
                    j = kb - 4 * cc
                    off = 128 * j if j > 0 else 0
                    out = {}
                    for h in range(2):
                        rows = slice(h * 64, (h + 1) * 64)
                        sps = pp.tile([128, _TC], f32, tag="sps", bufs=4)
                        nc.tensor.matmul(
                            sps[:, off:],
                            kTb[rows, kb * 128:(kb + 1) * 128],
                            qTb[rows, cc * _TC + off:(cc + 1) * _TC],
                            start=True, stop=True,
                        )
                        pexp = sp.tile([128, _TC], bf16, tag="pexp",
                                       bufs=30)
                        nc.scalar.activation(
                            pexp[:, off:], sps[:, off:], AF.Exp,
                            scale=0.125,
                        )
                        if j >= 0:
                            # diagonal 128-block: keep where q >= k,
                            # else fill exp(-10)
                            nc.gpsimd.affine_select(
                                out=pexp[:, 128 * j:128 * (j + 1)],
                                in_=pexp[:, 128 * j:128 * (j + 1)],
                                compare_op=mybir.AluOpType.is_ge,
                                fill=_EXPM,
                                base=0,
                                pattern=[[1, 128]],
                                channel_multiplier=-1,
                            )
                        out[h] = pexp
                    return out

                # chunk-pipelined: scores/exp for chunk c+1 are emitted
                # while chunk c's PV groups accumulate, so the exp chain
                # is a full chunk ahead of its consumers.  PSUM start
                # flags zero the whole 2KB bank, so each (h, u) group
                # must run contiguously: init + PVs + stop, one group
                # live per bank at a time.
                post = []
                pex = {kb: emit_score(0, kb) for kb in range(4)}
                for c in range(_NQC):
                    nkb = 4 * c + 4
                    nxt = list(range(nkb + 4)) if c < 3 else []
                    pex_next = {}
                    emitted = 0
                    # flush the previous chunk's deferred tail before its
                    # y2 tiles are re-acquired below
                    while post:
                        post.pop(0)()
                    y2 = []
                    for h in range(2):
                        t = pp.tile([128, 4, 128], f32, tag="y2", bufs=2,
                                    name=f"y2_{c}_{h}")
                        y2.append(t)
                    for u in range(4):
                        qt = 4 * c + u
                        for h in range(2):
                            nc.tensor.matmul(
                                y2[h][:, u, 0:65],
                                mbig[:, qt * 128:(qt + 1) * 128],
                                csum[:, h * 65:h * 65 + 65],
                                start=True, stop=False,
                            )
                            for kb in range(4 * c + u + 1):
                                nc.tensor.matmul(
                                    y2[h][:, u, 0:65],
                                    pex[kb][h][:, u * 128:(u + 1) * 128],
                                    vab[:, kb, h, :],
                                    start=False,
                                    stop=(kb == 4 * c + u),
                                )
                            share = (len(nxt) * (2 * u + h + 1)) // 8
                            while emitted < share:
                                pex_next[nxt[emitted]] = emit_score(
                                    c + 1, nxt[emitted])
                                emitted += 1
                            pump_w(2)
                        post.append(make_post(
                            b, c, u, y2,
                            b == _B - 1 and c == _NQC - 1))
                        if len(post) > 1:
                            post.pop(0)()
                        pump_w(1)
                    pex = pex_next
                    if bt_next is not None and c < 3:
                        push_chunk(4 * (b + 1) + c + 1, bt_next)
                while post:
                    post.pop(0)()

            # ---------------- main flow ----------------
            bt = batch_tiles()
            nc.sync.dma_start(w_sb["q"][:], wq_v[:])
            push_chunk(0, bt)
            load_consts()
            for t in range(1, 4):
                push_chunk(t, bt)
            drain()
            for b in range(_B):
                bt_next = batch_tiles() if b < _B - 1 else None
                if bt_next is not None:
                    push_chunk(4 * (b + 1), bt_next)
                attn_batch(b, bt, bt_next)
                # everything feeding attn(b+1) must be emitted before
                # attn(b+1) starts (in-order engines); normally a no-op
                drain()
                bt = bt_next
            drain()

    nc.compile()
    return nc, outT_d.name


def _get_nc():
    if "nc" not in _cache:
        _cache["nc"] = _build()
    return _cache["nc"]


def _consts():
    import ml_dtypes

    mbig = np.zeros((128, 16 * 128), dtype=np.float32)
    for kb in range(16):
        for qt in range(16):
            if kb > qt:
                mbig[kb, qt * 128:(qt + 1) * 128] = _EXPM
    onehot = np.zeros((128, 16 * 128), dtype=np.float32)
    for kb in range(16):
        onehot[:, kb * 128 + kb] = 1.0
    identr = np.eye(128, dtype=np.float32)
    return (
        mbig.astype(ml_dtypes.bfloat16),
        onehot.astype(ml_dtypes.bfloat16),
        identr,
    )


def kernel(**inputs):
    import ml_dtypes

    from concourse.bass_utils import run_bass_kernel_spmd

    x = np.ascontiguousarray(np.asarray(inputs["x"]), dtype=np.float32)
    wq = np.ascontiguousarray(np.asarray(inputs["wq"]), dtype=np.float32)
    wk = np.ascontiguousarray(np.asarray(inputs["wk"]), dtype=np.float32)
    wv = np.ascontiguousarray(np.asarray(inputs["wv"]), dtype=np.float32)
    wo = np.ascontiguousarray(np.asarray(inputs["wo"]), dtype=np.float32)

    bf16 = ml_dtypes.bfloat16
    xT = np.ascontiguousarray(x.reshape(_BT, _C).T).astype(bf16)
    mbig, onehot, identr = _consts()

    in_maps = []
    for i in range(_NC):
        r = slice(_LOC * i, _LOC * (i + 1))
        in_maps.append({
            "xT": xT,
            "wqT": np.ascontiguousarray(wq[r].T).astype(bf16),
            "wkT": np.ascontiguousarray(wk[r].T).astype(bf16),
            "wvT": np.ascontiguousarray(wv[r].T).astype(bf16),
            "woT": np.ascontiguousarray(wo[:, r].T).astype(bf16),
            "mbig": mbig,
            "onehot": onehot,
            "identr": identr,
        })

    nc, outname = _get_nc()
    try:
        res = run_bass_kernel_spmd(nc, in_maps, list(range(_NC)), trace=TRACE)
    except ModuleNotFoundError:
        # NTFF profiling hook unavailable in this container
        res = run_bass_kernel_spmd(nc, in_maps, list(range(_NC)), trace=False)

    global LAST_EXEC_NS, LAST_RESULTS
    LAST_EXEC_NS = res.exec_time_ns
    LAST_RESULTS = res

    acc = np.zeros((_BT, _C), dtype=np.float64)
    for i in range(_NC):
        acc += np.asarray(res.results[i][outname], dtype=np.float64)
    return acc.reshape(_B, _T, _C).astype(np.float32)
